# revision 6
# baseline (speedup 1.0000x reference)
"""Davis-Yin splitting LP solver kernel for Trainium2 (8 NeuronCores, data parallel).

Math per batch item (B=256 total, 32 per core):
  A = [As | I]  (128 x 640),  P = As_inv = pinv(A)  (640 x 128)
  iterate 50x:
    p2 = relu(s)
    t  = (2-a)*p2 - s - a*c
    r  = As @ t[:512] + t[512:] - b          (down-projection, 128)
    u  = As_inv @ r                          (up-projection, 640)
    s  = (s - p2) + t - u
  out = relu(s)

Device layout (per core):
  - State vectors in "column layout": SBUF [128 partitions, nb*5 cols],
    col (b*5+k) holds elements [128k : 128(k+1)) of item b's 640-vector.
  - Down-proj weights: AsT chunks, lhsT_k[dk, m] = As[m, 128k+dk] (4 per item).
  - Up-proj weights: Pinv chunks, lhsT_j[k, d'] = As_inv[128j+d', k] (5 per item).
  - All matvecs are PE matmuls with the matrix as the (self-loading fp32)
    stationary operand and an N=1 moving vector; elementwise work is batched
    across a half-group of items on ACT/DVE so it overlaps PE work.
"""

import numpy as np

import concourse.bass as bass
import concourse.mybir as mybir
from concourse.tile import TileContext
from concourse.bass_utils import run_bass_kernel_spmd

F32 = mybir.dt.float32
AF = mybir.ActivationFunctionType
ALU = mybir.AluOpType

B, M, N = 256, 128, 512
D = M + N  # 640
NCORES = 8
NB = B // NCORES  # 32 items per core
NUM_ITER = 50
ALPHA, TAU, DECAY = 0.05, 1.0, 10.0


def _alphas(num_iter):
    i = np.arange(num_iter, dtype=np.float32)
    base = np.float32(1.0) - i / np.float32(NUM_ITER)
    return (np.float32(ALPHA) * base ** (np.float32(1.0) / np.float32(DECAY))).astype(
        np.float32
    )


def _legalize_waits_json(raw: bytes) -> bytes:
    """Walrus (this revision) accepts at most 1 sync-wait per instruction
    (2 for EventSemaphore), but Tile emits up to 2 on compute instructions.
    Hoist excess waits onto standalone EventSemaphore instructions inserted
    just before the over-subscribed instruction (same engine, so the waits
    still happen-before it in queue order)."""
    import json as _json

    bir = _json.loads(raw)
    ctr = [0]

    def process_block(instrs):
        out = []
        for inst in instrs:
            si = inst.get("sync_info")
            if si:
                waits = si.get("on_wait") or []
                cap = 2 if inst.get("opcode") == "EventSemaphore" else 1
                if len(waits) > cap:
                    extra, keep = waits[:-cap], waits[-cap:]
                    for i in range(0, len(extra), 2):
                        ctr[0] += 1
                        out.append(
                            {
                                "debug": inst.get("debug", 0),
                                "engine": inst["engine"],
                                "ins": [],
                                "name": f"waitfix_{ctr[0]}",
                                "opcode": "EventSemaphore",
                                "outs": [],
                                "sync_info": {
                                    "on_update": [],
                                    "on_wait": extra[i : i + 2],
                                },
                            }
                        )
                    si["on_wait"] = keep
            out.append(inst)
        return out

    def walk(o):
        if isinstance(o, dict):
            for k, v in o.items():
                if k == "instructions" and isinstance(v, list):
                    o[k] = process_block(v)
                else:
                    walk(v)
        elif isinstance(o, list):
            for v in o:
                walk(v)

    walk(bir)
    return _json.dumps(bir).encode()


def _patch_serialization(nc):
    orig = nc.to_json_bytes

    def patched():
        return _legalize_waits_json(orig())

    nc.to_json_bytes = patched
    return nc


def build_program(nb=NB, num_iter=NUM_ITER, nh=2):
    """Build the per-core Bass program (identical across cores)."""
    nc = bass.Bass(use_seq_codegen=True)
    AsT_d = nc.dram_tensor("AsT", [nb, 4, 128, 128], F32, kind="ExternalInput")
    Pinv_d = nc.dram_tensor("Pinv", [nb, 5, 128, 128], F32, kind="ExternalInput")
    c_d = nc.dram_tensor("ccol", [128, nb * 5], F32, kind="ExternalInput")
    b_d = nc.dram_tensor("bcol", [128, nb], F32, kind="ExternalInput")
    out_d = nc.dram_tensor("out", [128, nb * 5], F32, kind="ExternalOutput")

    alphas = _alphas(num_iter)
    hs = nb // nh  # items per half-group

    with TileContext(nc) as tc:
        with (
            tc.tile_pool(name="wpool", bufs=1) as wpool,
            tc.tile_pool(name="spool", bufs=2) as spool,
            tc.tile_pool(name="tpool", bufs=2) as tpool,
            tc.tile_pool(name="ppool", bufs=2, space="PSUM") as ppool,
        ):
            AsT = wpool.tile([128, nb * 4 * 128], F32, tag="AsT")
            Pinv = wpool.tile([128, nb * 5 * 128], F32, tag="Pinv")
            ccol = wpool.tile([128, nb * 5], F32, tag="ccol")
            bcol = wpool.tile([128, nb], F32, tag="bcol")

            for b in range(nb):
                nc.sync.dma_start(
                    out=AsT[:, b * 512 : (b + 1) * 512].rearrange(
                        "p (k j) -> p k j", k=4
                    ),
                    in_=AsT_d[b].rearrange("k i j -> i k j"),
                )
                nc.sync.dma_start(
                    out=Pinv[:, b * 640 : (b + 1) * 640].rearrange(
                        "p (k j) -> p k j", k=5
                    ),
                    in_=Pinv_d[b].rearrange("k i j -> i k j"),
                )
            nc.sync.dma_start(out=ccol[:], in_=c_d[:])
            nc.sync.dma_start(out=bcol[:], in_=b_d[:])

            s = spool.tile([128, nb * 5], F32, tag="state")
            nc.gpsimd.memset(s[:], 0.0)

            for it in range(num_iter):
                a = float(alphas[it])
                s_new = spool.tile([128, nb * 5], F32, tag="state")
                for h in range(nh):
                    c0 = h * hs * 5  # first state col of this half
                    sl = slice(c0, c0 + hs * 5)
                    slb = slice(h * hs, (h + 1) * hs)
                    sh = s[:, sl]

                    p2s = tpool.tile([128, hs * 5], F32, tag="p2s")
                    mneg = tpool.tile([128, hs * 5], F32, tag="mneg")
                    t = tpool.tile([128, hs * 5], F32, tag="t")
                    w = tpool.tile([128, hs * 5], F32, tag="w")
                    tsb = tpool.tile([128, hs], F32, tag="tsb")
                    r = tpool.tile([128, hs], F32, tag="r")

                    # p2s = (2-a)*relu(s);  mneg = relu(-s)  (so s - p2 = -mneg)
                    nc.scalar.activation(p2s[:], sh, AF.Relu, scale=2.0 - a)
                    nc.scalar.activation(mneg[:], sh, AF.Relu, scale=-1.0)
                    # t = p2s - a*c - s
                    ac = tpool.tile([128, hs * 5], F32, tag="ac")
                    nc.vector.tensor_scalar_mul(ac[:], ccol[:, sl], -a)
                    nc.vector.tensor_add(t[:], ac[:], p2s[:])
                    nc.vector.tensor_sub(t[:], t[:], sh)
                    # w = t - mneg  (= s - p2 + t)
                    nc.vector.tensor_sub(w[:], t[:], mneg[:])
                    # tsb = t_slack - b
                    nc.vector.tensor_sub(tsb[:], t[:, 4::5], bcol[:, slb])

                    # down-projection: psum_y[:, bi] = As_b @ t_x
                    psum_y = ppool.tile([128, hs], F32, tag="py")
                    for bi in range(hs):
                        bg = h * hs + bi
                        for k in range(4):
                            nc.tensor.matmul(
                                psum_y[:, bi : bi + 1],
                                lhsT=AsT[:, (bg * 4 + k) * 128 : (bg * 4 + k + 1) * 128],
                                rhs=t[:, bi * 5 + k : bi * 5 + k + 1],
                                start=(k == 0),
                                stop=(k == 3),
                            )
                    # r = y + t_slack - b
                    nc.vector.tensor_add(r[:], psum_y[:], tsb[:])

                    # up-projection: psum_u[:, j*hs+bi] = As_inv chunk j @ r
                    psum_u = ppool.tile([128, 5 * hs], F32, tag="pu")
                    for bi in range(hs):
                        bg = h * hs + bi
                        for j in range(5):
                            nc.tensor.matmul(
                                psum_u[:, j * hs + bi : j * hs + bi + 1],
                                lhsT=Pinv[
                                    :, (bg * 5 + j) * 128 : (bg * 5 + j + 1) * 128
                                ],
                                rhs=r[:, bi : bi + 1],
                                start=True,
                                stop=True,
                            )
                    # s_new = w - u
                    for j in range(5):
                        nc.vector.tensor_sub(
                            s_new[:, c0 + j : c0 + hs * 5 : 5],
                            w[:, j::5],
                            psum_u[:, j * hs : (j + 1) * hs],
                        )
                s = s_new

            final = spool.tile([128, nb * 5], F32, tag="final")
            nc.scalar.activation(final[:], s[:], AF.Relu)
            nc.sync.dma_start(out=out_d[:], in_=final[:])

    return _patch_serialization(nc)


def _prep_core_inputs(c_input, As, bs, As_inv, nb):
    """Host-side marshaling of one core's shard into the device layouts."""
    AsT = np.ascontiguousarray(
        As.reshape(nb, 128, 4, 128).transpose(0, 2, 3, 1), dtype=np.float32
    )
    Pinv = np.ascontiguousarray(
        As_inv.reshape(nb, 5, 128, 128).transpose(0, 1, 3, 2), dtype=np.float32
    )
    ccol = np.ascontiguousarray(
        c_input.reshape(nb, 5, 128).transpose(2, 0, 1).reshape(128, nb * 5),
        dtype=np.float32,
    )
    bcol = np.ascontiguousarray(bs.T, dtype=np.float32)
    return {"AsT": AsT, "Pinv": Pinv, "ccol": ccol, "bcol": bcol}


def kernel(c_input, As, bs, As_inv, _trace=False, _nc_cache={}):
    c_input = np.asarray(c_input, dtype=np.float32)
    As = np.asarray(As, dtype=np.float32)
    bs = np.asarray(bs, dtype=np.float32)
    As_inv = np.asarray(As_inv, dtype=np.float32)

    if "nc" not in _nc_cache:
        _nc_cache["nc"] = build_program()
    nc = _nc_cache["nc"]

    in_maps = []
    for core in range(NCORES):
        sl = slice(core * NB, (core + 1) * NB)
        in_maps.append(_prep_core_inputs(c_input[sl], As[sl], bs[sl], As_inv[sl], NB))

    res = run_bass_kernel_spmd(nc, in_maps, core_ids=list(range(NCORES)), trace=_trace)

    out = np.empty((B, D), dtype=np.float32)
    for core in range(NCORES):
        oc = res.results[core]["out"]  # [128, NB*5]
        out[core * NB : (core + 1) * NB] = (
            oc.reshape(128, NB, 5).transpose(1, 2, 0).reshape(NB, D)
        )
    if _trace:
        kernel.last_exec_time_ns = res.exec_time_ns
    return out


# revision 12
# speedup vs baseline: 8.3911x; 8.3911x over previous
"""Davis-Yin splitting LP solver kernel for Trainium2 (8 NeuronCores, data parallel).

Math per batch item (B=256 total, 32 per core):
  A = [As | I]  (128 x 640),  P = As_inv = pinv(A)  (640 x 128)
  iterate 50x:
    p2 = relu(s)
    t  = (2-a)*p2 - s - a*c
    r  = As @ t[:512] + t[512:] - b          (down-projection, 128)
    u  = As_inv @ r                          (up-projection, 640)
    s  = (s - p2) + t - u
  out = relu(s)

Device layout (per core):
  - State vectors in "column layout": SBUF [128 partitions, nb*5 cols],
    col (b*5+k) holds elements [128k : 128(k+1)) of item b's 640-vector.
  - Down-proj weights: AsT chunks, lhsT_k[dk, m] = As[m, 128k+dk] (4 per item).
  - Up-proj weights: Pinv chunks, lhsT_j[k, d'] = As_inv[128j+d', k] (5 per item).
  - All matvecs are PE matmuls with the matrix as the (self-loading fp32)
    stationary operand and an N=1 moving vector; elementwise work is batched
    across a half-group of items on ACT/DVE so it overlaps PE work.
"""

import numpy as np

import concourse.bass as bass
import concourse.mybir as mybir
from concourse.tile import TileContext
from concourse.bass_utils import run_bass_kernel_spmd

F32 = mybir.dt.float32
AF = mybir.ActivationFunctionType
ALU = mybir.AluOpType

B, M, N = 256, 128, 512
D = M + N  # 640
NCORES = 8
NB = B // NCORES  # 32 items per core
NUM_ITER = 50
ALPHA, TAU, DECAY = 0.05, 1.0, 10.0


def _alphas(num_iter):
    i = np.arange(num_iter, dtype=np.float32)
    base = np.float32(1.0) - i / np.float32(NUM_ITER)
    return (np.float32(ALPHA) * base ** (np.float32(1.0) / np.float32(DECAY))).astype(
        np.float32
    )


def _legalize_waits_json(raw: bytes) -> bytes:
    """Walrus (this revision) accepts at most 1 sync-wait per instruction
    (2 for EventSemaphore), but Tile emits up to 2 on compute instructions.
    Hoist excess waits onto standalone EventSemaphore instructions inserted
    just before the over-subscribed instruction (same engine, so the waits
    still happen-before it in queue order)."""
    import json as _json

    bir = _json.loads(raw)
    ctr = [0]

    def process_block(instrs):
        out = []
        for inst in instrs:
            si = inst.get("sync_info")
            if si:
                waits = si.get("on_wait") or []
                cap = 2 if inst.get("opcode") == "EventSemaphore" else 1
                if len(waits) > cap:
                    extra, keep = waits[:-cap], waits[-cap:]
                    for i in range(0, len(extra), 2):
                        ctr[0] += 1
                        out.append(
                            {
                                "debug": inst.get("debug", 0),
                                "engine": inst["engine"],
                                "ins": [],
                                "name": f"waitfix_{ctr[0]}",
                                "opcode": "EventSemaphore",
                                "outs": [],
                                "sync_info": {
                                    "on_update": [],
                                    "on_wait": extra[i : i + 2],
                                },
                            }
                        )
                    si["on_wait"] = keep
            out.append(inst)
        return out

    def walk(o):
        if isinstance(o, dict):
            for k, v in o.items():
                if k == "instructions" and isinstance(v, list):
                    o[k] = process_block(v)
                else:
                    walk(v)
        elif isinstance(o, list):
            for v in o:
                walk(v)

    walk(bir)
    return _json.dumps(bir).encode()


def _patch_serialization(nc):
    orig = nc.to_json_bytes

    def patched():
        return _legalize_waits_json(orig())

    nc.to_json_bytes = patched
    return nc


def build_program(nb=NB, num_iter=NUM_ITER, nh=2, wdt=F32):
    """Build the per-core Bass program (identical across cores).

    wdt: dtype of the stationary matvec weights (fp32 or bf16). bf16 gets
    single-pass FWL weight loads (~4x faster PE) at ~1e-3 accuracy cost.
    """
    nc = bass.Bass(use_seq_codegen=True)
    AsT_d = nc.dram_tensor("AsT", [nb, 4, 128, 128], wdt, kind="ExternalInput")
    Pinv_d = nc.dram_tensor("Pinv", [nb, 5, 128, 128], wdt, kind="ExternalInput")
    c_d = nc.dram_tensor("ccol", [128, nb * 5], F32, kind="ExternalInput")
    b_d = nc.dram_tensor("bcol", [128, nb], F32, kind="ExternalInput")
    out_d = nc.dram_tensor("out", [128, nb * 5], F32, kind="ExternalOutput")

    alphas = _alphas(num_iter)
    hs = nb // nh  # items per half-group

    with TileContext(nc) as tc:
        with (
            tc.tile_pool(name="wpool", bufs=1) as wpool,
            tc.tile_pool(name="spool", bufs=2) as spool,
            tc.tile_pool(name="tpool", bufs=2) as tpool,
            tc.tile_pool(name="ppool", bufs=2, space="PSUM") as ppool,
        ):
            AsT = wpool.tile([128, nb * 4 * 128], wdt, tag="AsT")
            Pinv = wpool.tile([128, nb * 5 * 128], wdt, tag="Pinv")
            ccol = wpool.tile([128, nb * 5], F32, tag="ccol")
            bcol = wpool.tile([128, nb], F32, tag="bcol")

            for b in range(nb):
                nc.sync.dma_start(
                    out=AsT[:, b * 512 : (b + 1) * 512].rearrange(
                        "p (k j) -> p k j", k=4
                    ),
                    in_=AsT_d[b].rearrange("k i j -> i k j"),
                )
                nc.sync.dma_start(
                    out=Pinv[:, b * 640 : (b + 1) * 640].rearrange(
                        "p (k j) -> p k j", k=5
                    ),
                    in_=Pinv_d[b].rearrange("k i j -> i k j"),
                )
            nc.sync.dma_start(out=ccol[:], in_=c_d[:])
            nc.sync.dma_start(out=bcol[:], in_=b_d[:])

            s = spool.tile([128, nb * 5], F32, tag="state")
            nc.gpsimd.memset(s[:], 0.0)

            for it in range(num_iter):
                a = float(alphas[it])
                s_new = spool.tile([128, nb * 5], F32, tag="state")
                for h in range(nh):
                    c0 = h * hs * 5  # first state col of this half
                    sl = slice(c0, c0 + hs * 5)
                    slb = slice(h * hs, (h + 1) * hs)
                    sh = s[:, sl]

                    p2s = tpool.tile([128, hs * 5], F32, tag="p2s")
                    mneg = tpool.tile([128, hs * 5], F32, tag="mneg")
                    t = tpool.tile([128, hs * 5], F32, tag="t")
                    w = tpool.tile([128, hs * 5], F32, tag="w")
                    tsb = tpool.tile([128, hs], F32, tag="tsb")
                    r = tpool.tile([128, hs], F32, tag="r")

                    # p2s = (2-a)*relu(s);  mneg = relu(-s)  (so s - p2 = -mneg)
                    nc.scalar.activation(p2s[:], sh, AF.Relu, scale=2.0 - a)
                    nc.scalar.activation(mneg[:], sh, AF.Relu, scale=-1.0)
                    # t = p2s - a*c - s
                    ac = tpool.tile([128, hs * 5], F32, tag="ac")
                    nc.vector.tensor_scalar_mul(ac[:], ccol[:, sl], -a)
                    nc.vector.tensor_add(t[:], ac[:], p2s[:])
                    nc.vector.tensor_sub(t[:], t[:], sh)
                    # w = t - mneg  (= s - p2 + t)
                    nc.vector.tensor_sub(w[:], t[:], mneg[:])
                    # tsb = t_slack - b
                    nc.vector.tensor_sub(tsb[:], t[:, 4::5], bcol[:, slb])

                    if wdt != F32:
                        t_mm = tpool.tile([128, hs * 5], wdt, tag="tbf")
                        nc.vector.tensor_copy(t_mm[:], t[:])
                    else:
                        t_mm = t

                    # down-projection: psum_y[:, bi] = As_b @ t_x
                    psum_y = ppool.tile([128, hs], F32, tag="py")
                    for bi in range(hs):
                        bg = h * hs + bi
                        for k in range(4):
                            nc.tensor.matmul(
                                psum_y[:, bi : bi + 1],
                                lhsT=AsT[:, (bg * 4 + k) * 128 : (bg * 4 + k + 1) * 128],
                                rhs=t_mm[:, bi * 5 + k : bi * 5 + k + 1],
                                start=(k == 0),
                                stop=(k == 3),
                            )
                    # r = y + t_slack - b
                    nc.vector.tensor_add(r[:], psum_y[:], tsb[:])
                    if wdt != F32:
                        r_mm = tpool.tile([128, hs], wdt, tag="rbf")
                        nc.vector.tensor_copy(r_mm[:], r[:])
                    else:
                        r_mm = r

                    # up-projection: psum_u[:, j*hs+bi] = As_inv chunk j @ r
                    psum_u = ppool.tile([128, 5 * hs], F32, tag="pu")
                    for bi in range(hs):
                        bg = h * hs + bi
                        for j in range(5):
                            nc.tensor.matmul(
                                psum_u[:, j * hs + bi : j * hs + bi + 1],
                                lhsT=Pinv[
                                    :, (bg * 5 + j) * 128 : (bg * 5 + j + 1) * 128
                                ],
                                rhs=r_mm[:, bi : bi + 1],
                                start=True,
                                stop=True,
                            )
                    # s_new = w - u
                    for j in range(5):
                        nc.vector.tensor_sub(
                            s_new[:, c0 + j : c0 + hs * 5 : 5],
                            w[:, j::5],
                            psum_u[:, j * hs : (j + 1) * hs],
                        )
                s = s_new

            final = spool.tile([128, nb * 5], F32, tag="final")
            nc.scalar.activation(final[:], s[:], AF.Relu)
            nc.sync.dma_start(out=out_d[:], in_=final[:])

    return _patch_serialization(nc)


def _prep_core_inputs(c_input, As, bs, As_inv, nb, np_wdt=np.float32):
    """Host-side marshaling of one core's shard into the device layouts."""
    AsT = np.ascontiguousarray(
        As.reshape(nb, 128, 4, 128).transpose(0, 2, 3, 1)
    ).astype(np_wdt)
    Pinv = np.ascontiguousarray(
        As_inv.reshape(nb, 5, 128, 128).transpose(0, 1, 3, 2)
    ).astype(np_wdt)
    ccol = np.ascontiguousarray(
        c_input.reshape(nb, 5, 128).transpose(2, 0, 1).reshape(128, nb * 5),
        dtype=np.float32,
    )
    bcol = np.ascontiguousarray(bs.T, dtype=np.float32)
    return {"AsT": AsT, "Pinv": Pinv, "ccol": ccol, "bcol": bcol}


WEIGHT_DTYPE = "bf16"  # "f32" or "bf16"


def kernel(c_input, As, bs, As_inv, _trace=False, _nc_cache={}):
    import ml_dtypes

    c_input = np.asarray(c_input, dtype=np.float32)
    As = np.asarray(As, dtype=np.float32)
    bs = np.asarray(bs, dtype=np.float32)
    As_inv = np.asarray(As_inv, dtype=np.float32)

    wdt = mybir.dt.bfloat16 if WEIGHT_DTYPE == "bf16" else F32
    np_wdt = ml_dtypes.bfloat16 if WEIGHT_DTYPE == "bf16" else np.float32
    if "nc" not in _nc_cache:
        _nc_cache["nc"] = build_program(wdt=wdt)
    nc = _nc_cache["nc"]

    in_maps = []
    for core in range(NCORES):
        sl = slice(core * NB, (core + 1) * NB)
        in_maps.append(
            _prep_core_inputs(
                c_input[sl], As[sl], bs[sl], As_inv[sl], NB, np_wdt=np_wdt
            )
        )

    res = run_bass_kernel_spmd(nc, in_maps, core_ids=list(range(NCORES)), trace=_trace)

    out = np.empty((B, D), dtype=np.float32)
    for core in range(NCORES):
        oc = res.results[core]["out"]  # [128, NB*5]
        out[core * NB : (core + 1) * NB] = (
            oc.reshape(128, NB, 5).transpose(1, 2, 0).reshape(NB, D)
        )
    if _trace:
        kernel.last_exec_time_ns = res.exec_time_ns
    return out


# revision 13
# speedup vs baseline: 9.2688x; 1.1046x over previous
"""Davis-Yin splitting LP solver kernel for Trainium2 (8 NeuronCores, data parallel).

Math per batch item (B=256 total, 32 per core):
  A = [As | I]  (128 x 640),  P = As_inv = pinv(A)  (640 x 128)
  iterate 50x:
    p2 = relu(s)
    t  = (2-a)*p2 - s - a*c
    r  = As @ t[:512] + t[512:] - b          (down-projection, 128)
    u  = As_inv @ r                          (up-projection, 640)
    s  = (s - p2) + t - u
  out = relu(s)

Device layout (per core):
  - State vectors in "column layout": SBUF [128 partitions, nb*5 cols],
    col (b*5+k) holds elements [128k : 128(k+1)) of item b's 640-vector.
  - Down-proj weights: AsT chunks, lhsT_k[dk, m] = As[m, 128k+dk] (4 per item).
  - Up-proj weights: Pinv chunks, lhsT_j[k, d'] = As_inv[128j+d', k] (5 per item).
  - All matvecs are PE matmuls with the matrix as the (self-loading fp32)
    stationary operand and an N=1 moving vector; elementwise work is batched
    across a half-group of items on ACT/DVE so it overlaps PE work.
"""

import numpy as np

import concourse.bass as bass
import concourse.mybir as mybir
from concourse.tile import TileContext
from concourse.bass_utils import run_bass_kernel_spmd

F32 = mybir.dt.float32
AF = mybir.ActivationFunctionType
ALU = mybir.AluOpType

B, M, N = 256, 128, 512
D = M + N  # 640
NCORES = 8
NB = B // NCORES  # 32 items per core
NUM_ITER = 50
ALPHA, TAU, DECAY = 0.05, 1.0, 10.0


def _alphas(num_iter):
    i = np.arange(num_iter, dtype=np.float32)
    base = np.float32(1.0) - i / np.float32(NUM_ITER)
    return (np.float32(ALPHA) * base ** (np.float32(1.0) / np.float32(DECAY))).astype(
        np.float32
    )


def _legalize_waits_json(raw: bytes) -> bytes:
    """Walrus (this revision) accepts at most 1 sync-wait per instruction
    (2 for EventSemaphore), but Tile emits up to 2 on compute instructions.
    Hoist excess waits onto standalone EventSemaphore instructions inserted
    just before the over-subscribed instruction (same engine, so the waits
    still happen-before it in queue order)."""
    import json as _json

    bir = _json.loads(raw)
    ctr = [0]

    def process_block(instrs):
        out = []
        for inst in instrs:
            si = inst.get("sync_info")
            if si:
                waits = si.get("on_wait") or []
                cap = 2 if inst.get("opcode") == "EventSemaphore" else 1
                if len(waits) > cap:
                    extra, keep = waits[:-cap], waits[-cap:]
                    for i in range(0, len(extra), 2):
                        ctr[0] += 1
                        out.append(
                            {
                                "debug": inst.get("debug", 0),
                                "engine": inst["engine"],
                                "ins": [],
                                "name": f"waitfix_{ctr[0]}",
                                "opcode": "EventSemaphore",
                                "outs": [],
                                "sync_info": {
                                    "on_update": [],
                                    "on_wait": extra[i : i + 2],
                                },
                            }
                        )
                    si["on_wait"] = keep
            out.append(inst)
        return out

    def walk(o):
        if isinstance(o, dict):
            for k, v in o.items():
                if k == "instructions" and isinstance(v, list):
                    o[k] = process_block(v)
                else:
                    walk(v)
        elif isinstance(o, list):
            for v in o:
                walk(v)

    walk(bir)
    return _json.dumps(bir).encode()


def _patch_serialization(nc):
    orig = nc.to_json_bytes

    def patched():
        return _legalize_waits_json(orig())

    nc.to_json_bytes = patched
    return nc


def build_program(nb=NB, num_iter=NUM_ITER, nh=2, wdt=F32):
    """Build the per-core Bass program (identical across cores).

    wdt: dtype of the stationary matvec weights (fp32 or bf16). bf16 gets
    single-pass FWL weight loads (~4x faster PE) at ~1e-3 accuracy cost.
    """
    nc = bass.Bass(use_seq_codegen=True)
    AsT_d = nc.dram_tensor("AsT", [nb, 4, 128, 128], wdt, kind="ExternalInput")
    Pinv_d = nc.dram_tensor("Pinv", [nb, 5, 128, 128], wdt, kind="ExternalInput")
    c_d = nc.dram_tensor("ccol", [128, nb * 5], F32, kind="ExternalInput")
    b_d = nc.dram_tensor("bcol", [128, nb], F32, kind="ExternalInput")
    out_d = nc.dram_tensor("out", [128, nb * 5], F32, kind="ExternalOutput")

    alphas = _alphas(num_iter)
    hs = nb // nh  # items per half-group

    with TileContext(nc) as tc:
        with (
            tc.tile_pool(name="wpool", bufs=1) as wpool,
            tc.tile_pool(name="spool", bufs=2) as spool,
            tc.tile_pool(name="tpool", bufs=2) as tpool,
            tc.tile_pool(name="ppool", bufs=2, space="PSUM") as ppool,
        ):
            AsT = wpool.tile([128, nb * 4 * 128], wdt, tag="AsT")
            Pinv = wpool.tile([128, nb * 5 * 128], wdt, tag="Pinv")
            ccol = wpool.tile([128, nb * 5], F32, tag="ccol")
            bcol = wpool.tile([128, nb], F32, tag="bcol")

            for b in range(nb):
                nc.sync.dma_start(
                    out=AsT[:, b * 512 : (b + 1) * 512].rearrange(
                        "p (k j) -> p k j", k=4
                    ),
                    in_=AsT_d[b].rearrange("k i j -> i k j"),
                )
                nc.sync.dma_start(
                    out=Pinv[:, b * 640 : (b + 1) * 640].rearrange(
                        "p (k j) -> p k j", k=5
                    ),
                    in_=Pinv_d[b].rearrange("k i j -> i k j"),
                )
            nc.sync.dma_start(out=ccol[:], in_=c_d[:])
            nc.sync.dma_start(out=bcol[:], in_=b_d[:])

            # Per-half state tiles keep the half-group pipelines independent
            # across iterations (a shared tile would serialize them).
            states = []
            for h in range(nh):
                sh0 = spool.tile([128, hs * 5], F32, tag=f"state{h}")
                nc.gpsimd.memset(sh0[:], 0.0)
                states.append(sh0)

            for it in range(num_iter):
                a = float(alphas[it])
                for h in range(nh):
                    c0 = h * hs * 5  # first state col of this half
                    sl = slice(c0, c0 + hs * 5)
                    slb = slice(h * hs, (h + 1) * hs)
                    sh = states[h]

                    p2s = tpool.tile([128, hs * 5], F32, tag=f"p2s{h}")
                    mneg = tpool.tile([128, hs * 5], F32, tag=f"mneg{h}")
                    q = tpool.tile([128, hs * 5], F32, tag=f"q{h}")
                    t = tpool.tile([128, hs * 5], F32, tag=f"t{h}")
                    w = tpool.tile([128, hs * 5], F32, tag=f"w{h}")
                    tsb = tpool.tile([128, hs], F32, tag=f"tsb{h}")

                    # p2s = (2-a)*relu(s);  mneg = relu(-s)  (so s - p2 = -mneg)
                    nc.scalar.activation(p2s[:], sh[:], AF.Relu, scale=2.0 - a)
                    nc.scalar.activation(mneg[:], sh[:], AF.Relu, scale=-1.0)
                    # t = p2s - (a*c + s)
                    nc.vector.scalar_tensor_tensor(
                        q[:], ccol[:, sl], a, sh[:], op0=ALU.mult, op1=ALU.add
                    )
                    nc.vector.tensor_sub(t[:], p2s[:], q[:])
                    if wdt != F32:
                        t_mm = tpool.tile([128, hs * 5], wdt, tag=f"tbf{h}")
                        nc.vector.tensor_copy(t_mm[:], t[:])
                    else:
                        t_mm = t
                    # tsb = t_slack - b;  w = t - mneg (= s - p2 + t)
                    nc.vector.tensor_sub(tsb[:], t[:, 4::5], bcol[:, slb])
                    nc.vector.tensor_sub(w[:], t[:], mneg[:])

                    # down-projection: psum_y[:, bi] = As_b @ t_x
                    psum_y = ppool.tile([128, hs], F32, tag=f"py{h}")
                    for bi in range(hs):
                        bg = h * hs + bi
                        for k in range(4):
                            nc.tensor.matmul(
                                psum_y[:, bi : bi + 1],
                                lhsT=AsT[:, (bg * 4 + k) * 128 : (bg * 4 + k + 1) * 128],
                                rhs=t_mm[:, bi * 5 + k : bi * 5 + k + 1],
                                start=(k == 0),
                                stop=(k == 3),
                            )
                    # r = y + t_slack - b  (cast to weight dtype fused)
                    r_mm = tpool.tile([128, hs], wdt, tag=f"rbf{h}")
                    nc.vector.tensor_add(r_mm[:], psum_y[:], tsb[:])

                    # up-projection: psum_u[:, bi*5+j] = As_inv chunk j @ r
                    psum_u = ppool.tile([128, 5 * hs], F32, tag=f"pu{h}")
                    for bi in range(hs):
                        bg = h * hs + bi
                        for j in range(5):
                            nc.tensor.matmul(
                                psum_u[:, bi * 5 + j : bi * 5 + j + 1],
                                lhsT=Pinv[
                                    :, (bg * 5 + j) * 128 : (bg * 5 + j + 1) * 128
                                ],
                                rhs=r_mm[:, bi : bi + 1],
                                start=True,
                                stop=True,
                            )
                    # s_new = w - u   (single op: psum_u columns match w layout)
                    s_new = spool.tile([128, hs * 5], F32, tag=f"state{h}")
                    nc.vector.tensor_sub(s_new[:], w[:], psum_u[:])
                    states[h] = s_new

            final = wpool.tile([128, nb * 5], F32, tag="final")
            for h in range(nh):
                nc.scalar.activation(
                    final[:, h * hs * 5 : (h + 1) * hs * 5], states[h][:], AF.Relu
                )
            nc.sync.dma_start(out=out_d[:], in_=final[:])

    return _patch_serialization(nc)


def _prep_core_inputs(c_input, As, bs, As_inv, nb, np_wdt=np.float32):
    """Host-side marshaling of one core's shard into the device layouts."""
    AsT = np.ascontiguousarray(
        As.reshape(nb, 128, 4, 128).transpose(0, 2, 3, 1)
    ).astype(np_wdt)
    Pinv = np.ascontiguousarray(
        As_inv.reshape(nb, 5, 128, 128).transpose(0, 1, 3, 2)
    ).astype(np_wdt)
    ccol = np.ascontiguousarray(
        c_input.reshape(nb, 5, 128).transpose(2, 0, 1).reshape(128, nb * 5),
        dtype=np.float32,
    )
    bcol = np.ascontiguousarray(bs.T, dtype=np.float32)
    return {"AsT": AsT, "Pinv": Pinv, "ccol": ccol, "bcol": bcol}


WEIGHT_DTYPE = "bf16"  # "f32" or "bf16"


def kernel(c_input, As, bs, As_inv, _trace=False, _nc_cache={}):
    import ml_dtypes

    c_input = np.asarray(c_input, dtype=np.float32)
    As = np.asarray(As, dtype=np.float32)
    bs = np.asarray(bs, dtype=np.float32)
    As_inv = np.asarray(As_inv, dtype=np.float32)

    wdt = mybir.dt.bfloat16 if WEIGHT_DTYPE == "bf16" else F32
    np_wdt = ml_dtypes.bfloat16 if WEIGHT_DTYPE == "bf16" else np.float32
    if "nc" not in _nc_cache:
        _nc_cache["nc"] = build_program(wdt=wdt)
    nc = _nc_cache["nc"]

    in_maps = []
    for core in range(NCORES):
        sl = slice(core * NB, (core + 1) * NB)
        in_maps.append(
            _prep_core_inputs(
                c_input[sl], As[sl], bs[sl], As_inv[sl], NB, np_wdt=np_wdt
            )
        )

    res = run_bass_kernel_spmd(nc, in_maps, core_ids=list(range(NCORES)), trace=_trace)

    out = np.empty((B, D), dtype=np.float32)
    for core in range(NCORES):
        oc = res.results[core]["out"]  # [128, NB*5]
        out[core * NB : (core + 1) * NB] = (
            oc.reshape(128, NB, 5).transpose(1, 2, 0).reshape(NB, D)
        )
    if _trace:
        kernel.last_exec_time_ns = res.exec_time_ns
    return out


# revision 15
# speedup vs baseline: 9.5687x; 1.0323x over previous
"""Davis-Yin splitting LP solver kernel for Trainium2 (8 NeuronCores, data parallel).

Math per batch item (B=256 total, 32 per core):
  A = [As | I]  (128 x 640),  P = As_inv = pinv(A)  (640 x 128)
  iterate 50x:
    p2 = relu(s)
    t  = (2-a)*p2 - s - a*c
    r  = As @ t[:512] + t[512:] - b          (down-projection, 128)
    u  = As_inv @ r                          (up-projection, 640)
    s  = (s - p2) + t - u
  out = relu(s)

Device layout (per core):
  - State vectors in "column layout": SBUF [128 partitions, nb*5 cols],
    col (b*5+k) holds elements [128k : 128(k+1)) of item b's 640-vector.
  - Down-proj weights: AsT chunks, lhsT_k[dk, m] = As[m, 128k+dk] (4 per item).
  - Up-proj weights: Pinv chunks, lhsT_j[k, d'] = As_inv[128j+d', k] (5 per item).
  - All matvecs are PE matmuls with the matrix as the (self-loading fp32)
    stationary operand and an N=1 moving vector; elementwise work is batched
    across a half-group of items on ACT/DVE so it overlaps PE work.
"""

import numpy as np

import concourse.bass as bass
import concourse.mybir as mybir
from concourse.tile import TileContext
from concourse.bass_utils import run_bass_kernel_spmd

F32 = mybir.dt.float32
AF = mybir.ActivationFunctionType
ALU = mybir.AluOpType

B, M, N = 256, 128, 512
D = M + N  # 640
NCORES = 8
NB = B // NCORES  # 32 items per core
NUM_ITER = 50
ALPHA, TAU, DECAY = 0.05, 1.0, 10.0


def _alphas(num_iter):
    i = np.arange(num_iter, dtype=np.float32)
    base = np.float32(1.0) - i / np.float32(NUM_ITER)
    return (np.float32(ALPHA) * base ** (np.float32(1.0) / np.float32(DECAY))).astype(
        np.float32
    )


def _legalize_waits_json(raw: bytes) -> bytes:
    """Walrus (this revision) accepts at most 1 sync-wait per instruction
    (2 for EventSemaphore), but Tile emits up to 2 on compute instructions.
    Hoist excess waits onto standalone EventSemaphore instructions inserted
    just before the over-subscribed instruction (same engine, so the waits
    still happen-before it in queue order)."""
    import json as _json

    bir = _json.loads(raw)
    ctr = [0]

    def process_block(instrs):
        out = []
        for inst in instrs:
            si = inst.get("sync_info")
            if si:
                waits = si.get("on_wait") or []
                cap = 2 if inst.get("opcode") == "EventSemaphore" else 1
                if len(waits) > cap:
                    extra, keep = waits[:-cap], waits[-cap:]
                    for i in range(0, len(extra), 2):
                        ctr[0] += 1
                        out.append(
                            {
                                "debug": inst.get("debug", 0),
                                "engine": inst["engine"],
                                "ins": [],
                                "name": f"waitfix_{ctr[0]}",
                                "opcode": "EventSemaphore",
                                "outs": [],
                                "sync_info": {
                                    "on_update": [],
                                    "on_wait": extra[i : i + 2],
                                },
                            }
                        )
                    si["on_wait"] = keep
            out.append(inst)
        return out

    def walk(o):
        if isinstance(o, dict):
            for k, v in o.items():
                if k == "instructions" and isinstance(v, list):
                    o[k] = process_block(v)
                else:
                    walk(v)
        elif isinstance(o, list):
            for v in o:
                walk(v)

    walk(bir)
    return _json.dumps(bir).encode()


def _patch_serialization(nc):
    orig = nc.to_json_bytes

    def patched():
        return _legalize_waits_json(orig())

    nc.to_json_bytes = patched
    return nc


def build_program(nb=NB, num_iter=NUM_ITER, nh=2, wdt=F32):
    """Build the per-core Bass program (identical across cores).

    wdt: dtype of the stationary matvec weights (fp32 or bf16). bf16 gets
    single-pass FWL weight loads (~4x faster PE) at ~1e-3 accuracy cost.
    """
    nc = bass.Bass(use_seq_codegen=True)
    AsT_d = nc.dram_tensor("AsT", [nb, 4, 128, 128], wdt, kind="ExternalInput")
    Pinv_d = nc.dram_tensor("Pinv", [nb, 5, 128, 128], wdt, kind="ExternalInput")
    c_d = nc.dram_tensor("ccol", [128, nb * 5], F32, kind="ExternalInput")
    b_d = nc.dram_tensor("bcol", [128, nb], F32, kind="ExternalInput")
    out_d = nc.dram_tensor("out", [128, nb * 5], F32, kind="ExternalOutput")

    alphas = _alphas(num_iter)
    hs = nb // nh  # items per half-group

    with TileContext(nc) as tc:
        with (
            tc.tile_pool(name="wpool", bufs=1) as wpool,
            tc.tile_pool(name="spool", bufs=2) as spool,
            tc.tile_pool(name="tpool", bufs=2) as tpool,
            tc.tile_pool(name="ppool", bufs=2, space="PSUM") as ppool,
        ):
            # Per-item weight tiles: item b's first matmul only waits for its
            # own DMA, not the whole 9.5MB load.
            AsT_t, Pinv_t = [], []
            ccol = wpool.tile([128, nb * 5], F32, tag="ccol")
            bcol = wpool.tile([128, nb], F32, tag="bcol")
            nc.sync.dma_start(out=ccol[:], in_=c_d[:])
            nc.sync.dma_start(out=bcol[:], in_=b_d[:])
            for b in range(nb):
                at = wpool.tile([128, 4 * 128], wdt, tag=f"AsT{b}")
                pv = wpool.tile([128, 5 * 128], wdt, tag=f"Pinv{b}")
                nc.sync.dma_start(
                    out=at[:].rearrange("p (k j) -> p k j", k=4),
                    in_=AsT_d[b].rearrange("k i j -> i k j"),
                )
                nc.sync.dma_start(
                    out=pv[:].rearrange("p (k j) -> p k j", k=5),
                    in_=Pinv_d[b].rearrange("k i j -> i k j"),
                )
                AsT_t.append(at)
                Pinv_t.append(pv)

            # Software pipeline: the elementwise "prep" for half h's iteration
            # i+1 (t, t_mm, tsb, w) is emitted right after its s_new, so it
            # runs on DVE/ACT while the PE chews the other halves' matmuls.
            def emit_prep(h, sh, a):
                sl = slice(h * hs * 5, (h + 1) * hs * 5)
                slb = slice(h * hs, (h + 1) * hs)
                p2s = tpool.tile([128, hs * 5], F32, tag=f"p2s{h}")
                mneg = tpool.tile([128, hs * 5], F32, tag=f"mneg{h}")
                q = tpool.tile([128, hs * 5], F32, tag=f"q{h}")
                t = tpool.tile([128, hs * 5], F32, tag=f"t{h}")
                w = tpool.tile([128, hs * 5], F32, tag=f"w{h}")
                tsb = tpool.tile([128, hs], F32, tag=f"tsb{h}")

                # p2s = (2-a)*relu(s);  mneg = relu(-s)  (so s - p2 = -mneg)
                nc.scalar.activation(p2s[:], sh[:], AF.Relu, scale=2.0 - a)
                nc.scalar.activation(mneg[:], sh[:], AF.Relu, scale=-1.0)
                # t = p2s - (a*c + s)
                nc.vector.scalar_tensor_tensor(
                    q[:], ccol[:, sl], a, sh[:], op0=ALU.mult, op1=ALU.add
                )
                nc.vector.tensor_sub(t[:], p2s[:], q[:])
                if wdt != F32:
                    t_mm = tpool.tile([128, hs * 5], wdt, tag=f"tbf{h}")
                    nc.vector.tensor_copy(t_mm[:], t[:])
                else:
                    t_mm = t
                # tsb = t_slack - b;  w = t - mneg (= s - p2 + t)
                nc.vector.tensor_sub(tsb[:], t[:, 4::5], bcol[:, slb])
                nc.vector.tensor_sub(w[:], t[:], mneg[:])
                return t_mm, tsb, w

            states, preps = [], []
            for h in range(nh):
                sh0 = spool.tile([128, hs * 5], F32, tag=f"state{h}")
                nc.gpsimd.memset(sh0[:], 0.0)
                states.append(sh0)
                preps.append(emit_prep(h, sh0, float(alphas[0])))

            for it in range(num_iter):
                for h in range(nh):
                    t_mm, tsb, w = preps[h]

                    # down-projection: psum_y[:, bi] = As_b @ t_x
                    psum_y = ppool.tile([128, hs], F32, tag=f"py{h}")
                    for bi in range(hs):
                        bg = h * hs + bi
                        for k in range(4):
                            nc.tensor.matmul(
                                psum_y[:, bi : bi + 1],
                                lhsT=AsT_t[bg][:, k * 128 : (k + 1) * 128],
                                rhs=t_mm[:, bi * 5 + k : bi * 5 + k + 1],
                                start=(k == 0),
                                stop=(k == 3),
                            )
                    # r = y + t_slack - b  (cast to weight dtype fused)
                    r_mm = tpool.tile([128, hs], wdt, tag=f"rbf{h}")
                    nc.vector.tensor_add(r_mm[:], psum_y[:], tsb[:])

                    # up-projection: psum_u[:, bi*5+j] = As_inv chunk j @ r
                    psum_u = ppool.tile([128, 5 * hs], F32, tag=f"pu{h}")
                    for bi in range(hs):
                        bg = h * hs + bi
                        for j in range(5):
                            nc.tensor.matmul(
                                psum_u[:, bi * 5 + j : bi * 5 + j + 1],
                                lhsT=Pinv_t[bg][:, j * 128 : (j + 1) * 128],
                                rhs=r_mm[:, bi : bi + 1],
                                start=True,
                                stop=True,
                            )
                    # s_new = w - u   (single op: psum_u columns match w layout)
                    s_new = spool.tile([128, hs * 5], F32, tag=f"state{h}")
                    nc.vector.tensor_sub(s_new[:], w[:], psum_u[:])
                    states[h] = s_new
                    if it + 1 < num_iter:
                        preps[h] = emit_prep(h, s_new, float(alphas[it + 1]))

            final = wpool.tile([128, nb * 5], F32, tag="final")
            for h in range(nh):
                nc.scalar.activation(
                    final[:, h * hs * 5 : (h + 1) * hs * 5], states[h][:], AF.Relu
                )
            nc.sync.dma_start(out=out_d[:], in_=final[:])

    return _patch_serialization(nc)


def _prep_core_inputs(c_input, As, bs, As_inv, nb, np_wdt=np.float32):
    """Host-side marshaling of one core's shard into the device layouts."""
    AsT = np.ascontiguousarray(
        As.reshape(nb, 128, 4, 128).transpose(0, 2, 3, 1)
    ).astype(np_wdt)
    Pinv = np.ascontiguousarray(
        As_inv.reshape(nb, 5, 128, 128).transpose(0, 1, 3, 2)
    ).astype(np_wdt)
    ccol = np.ascontiguousarray(
        c_input.reshape(nb, 5, 128).transpose(2, 0, 1).reshape(128, nb * 5),
        dtype=np.float32,
    )
    bcol = np.ascontiguousarray(bs.T, dtype=np.float32)
    return {"AsT": AsT, "Pinv": Pinv, "ccol": ccol, "bcol": bcol}


WEIGHT_DTYPE = "bf16"  # "f32" or "bf16"


def kernel(c_input, As, bs, As_inv, _trace=False, _nc_cache={}):
    import ml_dtypes

    c_input = np.asarray(c_input, dtype=np.float32)
    As = np.asarray(As, dtype=np.float32)
    bs = np.asarray(bs, dtype=np.float32)
    As_inv = np.asarray(As_inv, dtype=np.float32)

    wdt = mybir.dt.bfloat16 if WEIGHT_DTYPE == "bf16" else F32
    np_wdt = ml_dtypes.bfloat16 if WEIGHT_DTYPE == "bf16" else np.float32
    if "nc" not in _nc_cache:
        _nc_cache["nc"] = build_program(wdt=wdt)
    nc = _nc_cache["nc"]

    in_maps = []
    for core in range(NCORES):
        sl = slice(core * NB, (core + 1) * NB)
        in_maps.append(
            _prep_core_inputs(
                c_input[sl], As[sl], bs[sl], As_inv[sl], NB, np_wdt=np_wdt
            )
        )

    res = run_bass_kernel_spmd(nc, in_maps, core_ids=list(range(NCORES)), trace=_trace)

    out = np.empty((B, D), dtype=np.float32)
    for core in range(NCORES):
        oc = res.results[core]["out"]  # [128, NB*5]
        out[core * NB : (core + 1) * NB] = (
            oc.reshape(128, NB, 5).transpose(1, 2, 0).reshape(NB, D)
        )
    if _trace:
        kernel.last_exec_time_ns = res.exec_time_ns
    return out


# revision 16
# speedup vs baseline: 10.8809x; 1.1371x over previous
"""Davis-Yin splitting LP solver kernel for Trainium2 (8 NeuronCores, data parallel).

Math per batch item (B=256 total, 32 per core):
  A = [As | I]  (128 x 640),  P = As_inv = pinv(A)  (640 x 128)
  iterate 50x:
    p2 = relu(s)
    t  = (2-a)*p2 - s - a*c
    r  = As @ t[:512] + t[512:] - b          (down-projection, 128)
    u  = As_inv @ r                          (up-projection, 640)
    s  = (s - p2) + t - u
  out = relu(s)

Device layout (per core):
  - State vectors in "column layout": SBUF [128 partitions, nb*5 cols],
    col (b*5+k) holds elements [128k : 128(k+1)) of item b's 640-vector.
  - Down-proj weights: AsT chunks, lhsT_k[dk, m] = As[m, 128k+dk] (4 per item).
  - Up-proj weights: Pinv chunks, lhsT_j[k, d'] = As_inv[128j+d', k] (5 per item).
  - All matvecs are PE matmuls with the matrix as the (self-loading fp32)
    stationary operand and an N=1 moving vector; elementwise work is batched
    across a half-group of items on ACT/DVE so it overlaps PE work.
"""

import numpy as np

import concourse.bass as bass
import concourse.mybir as mybir
from concourse.tile import TileContext
from concourse.bass_utils import run_bass_kernel_spmd

F32 = mybir.dt.float32
AF = mybir.ActivationFunctionType
ALU = mybir.AluOpType

B, M, N = 256, 128, 512
D = M + N  # 640
NCORES = 8
NB = B // NCORES  # 32 items per core
NUM_ITER = 50
ALPHA, TAU, DECAY = 0.05, 1.0, 10.0


def _alphas(num_iter):
    i = np.arange(num_iter, dtype=np.float32)
    base = np.float32(1.0) - i / np.float32(NUM_ITER)
    return (np.float32(ALPHA) * base ** (np.float32(1.0) / np.float32(DECAY))).astype(
        np.float32
    )


def _legalize_waits_json(raw: bytes) -> bytes:
    """Walrus (this revision) accepts at most 1 sync-wait per instruction
    (2 for EventSemaphore), but Tile emits up to 2 on compute instructions.
    Hoist excess waits onto standalone EventSemaphore instructions inserted
    just before the over-subscribed instruction (same engine, so the waits
    still happen-before it in queue order)."""
    import json as _json

    bir = _json.loads(raw)
    ctr = [0]

    def process_block(instrs):
        out = []
        for inst in instrs:
            si = inst.get("sync_info")
            if si:
                waits = si.get("on_wait") or []
                cap = 2 if inst.get("opcode") == "EventSemaphore" else 1
                if len(waits) > cap:
                    extra, keep = waits[:-cap], waits[-cap:]
                    for i in range(0, len(extra), 2):
                        ctr[0] += 1
                        out.append(
                            {
                                "debug": inst.get("debug", 0),
                                "engine": inst["engine"],
                                "ins": [],
                                "name": f"waitfix_{ctr[0]}",
                                "opcode": "EventSemaphore",
                                "outs": [],
                                "sync_info": {
                                    "on_update": [],
                                    "on_wait": extra[i : i + 2],
                                },
                            }
                        )
                    si["on_wait"] = keep
            out.append(inst)
        return out

    def walk(o):
        if isinstance(o, dict):
            for k, v in o.items():
                if k == "instructions" and isinstance(v, list):
                    o[k] = process_block(v)
                else:
                    walk(v)
        elif isinstance(o, list):
            for v in o:
                walk(v)

    walk(bir)
    return _json.dumps(bir).encode()


def _patch_serialization(nc):
    orig = nc.to_json_bytes

    def patched():
        return _legalize_waits_json(orig())

    nc.to_json_bytes = patched
    return nc


def build_program(nb=NB, num_iter=NUM_ITER, nh=4, wdt=F32):
    """Build the per-core Bass program (identical across cores).

    wdt: dtype of the stationary matvec weights (fp32 or bf16). bf16 gets
    single-pass FWL weight loads (~4x faster PE) at ~1e-3 accuracy cost.
    """
    nc = bass.Bass(use_seq_codegen=True)
    AsT_d = nc.dram_tensor("AsT", [nb, 4, 128, 128], wdt, kind="ExternalInput")
    Pinv_d = nc.dram_tensor("Pinv", [nb, 5, 128, 128], wdt, kind="ExternalInput")
    c_d = nc.dram_tensor("ccol", [128, nb * 5], F32, kind="ExternalInput")
    b_d = nc.dram_tensor("bcol", [128, nb], F32, kind="ExternalInput")
    out_d = nc.dram_tensor("out", [128, nb * 5], F32, kind="ExternalOutput")

    alphas = _alphas(num_iter)
    hs = nb // nh  # items per half-group

    with TileContext(nc) as tc:
        with (
            tc.tile_pool(name="wpool", bufs=1) as wpool,
            tc.tile_pool(name="spool", bufs=2) as spool,
            tc.tile_pool(name="tpool", bufs=2) as tpool,
            tc.tile_pool(name="ppool", bufs=1, space="PSUM") as ppool,
        ):
            # Per-item weight tiles: item b's first matmul only waits for its
            # own DMA, not the whole 9.5MB load.
            AsT_t, Pinv_t = [], []
            ccol = wpool.tile([128, nb * 5], F32, tag="ccol")
            bcol = wpool.tile([128, nb], F32, tag="bcol")
            nc.sync.dma_start(out=ccol[:], in_=c_d[:])
            nc.sync.dma_start(out=bcol[:], in_=b_d[:])
            for b in range(nb):
                at = wpool.tile([128, 4 * 128], wdt, tag=f"AsT{b}")
                pv = wpool.tile([128, 5 * 128], wdt, tag=f"Pinv{b}")
                nc.sync.dma_start(
                    out=at[:].rearrange("p (k j) -> p k j", k=4),
                    in_=AsT_d[b].rearrange("k i j -> i k j"),
                )
                nc.sync.dma_start(
                    out=pv[:].rearrange("p (k j) -> p k j", k=5),
                    in_=Pinv_d[b].rearrange("k i j -> i k j"),
                )
                AsT_t.append(at)
                Pinv_t.append(pv)

            # Software pipeline: the elementwise "prep" for half h's iteration
            # i+1 (t, t_mm, tsb, w) is emitted right after its s_new, so it
            # runs on DVE/ACT while the PE chews the other halves' matmuls.
            def emit_prep(h, sh, a):
                sl = slice(h * hs * 5, (h + 1) * hs * 5)
                slb = slice(h * hs, (h + 1) * hs)
                p2s = tpool.tile([128, hs * 5], F32, tag=f"p2s{h}")
                mneg = tpool.tile([128, hs * 5], F32, tag=f"mneg{h}")
                q = tpool.tile([128, hs * 5], F32, tag=f"q{h}")
                t = tpool.tile([128, hs * 5], F32, tag=f"t{h}")
                w = tpool.tile([128, hs * 5], F32, tag=f"w{h}")
                tsb = tpool.tile([128, hs], F32, tag=f"tsb{h}")

                # p2s = (2-a)*relu(s);  mneg = relu(-s)  (so s - p2 = -mneg)
                nc.scalar.activation(p2s[:], sh[:], AF.Relu, scale=2.0 - a)
                nc.scalar.activation(mneg[:], sh[:], AF.Relu, scale=-1.0)
                # t = p2s - (a*c + s)
                nc.vector.scalar_tensor_tensor(
                    q[:], ccol[:, sl], a, sh[:], op0=ALU.mult, op1=ALU.add
                )
                nc.vector.tensor_sub(t[:], p2s[:], q[:])
                if wdt != F32:
                    t_mm = tpool.tile([128, hs * 5], wdt, tag=f"tbf{h}")
                    nc.vector.tensor_copy(t_mm[:], t[:])
                else:
                    t_mm = t
                # tsb = t_slack - b;  w = t - mneg (= s - p2 + t)
                nc.vector.tensor_sub(tsb[:], t[:, 4::5], bcol[:, slb])
                nc.vector.tensor_sub(w[:], t[:], mneg[:])
                return t_mm, tsb, w

            states, preps = [], []
            for h in range(nh):
                sh0 = spool.tile([128, hs * 5], F32, tag=f"state{h}")
                nc.gpsimd.memset(sh0[:], 0.0)
                states.append(sh0)
                preps.append(emit_prep(h, sh0, float(alphas[0])))

            for it in range(num_iter):
                for h in range(nh):
                    t_mm, tsb, w = preps[h]

                    # down-projection: psum_y[:, bi] = As_b @ t_x
                    psum_y = ppool.tile([128, hs], F32, tag=f"py{h}")
                    for bi in range(hs):
                        bg = h * hs + bi
                        for k in range(4):
                            nc.tensor.matmul(
                                psum_y[:, bi : bi + 1],
                                lhsT=AsT_t[bg][:, k * 128 : (k + 1) * 128],
                                rhs=t_mm[:, bi * 5 + k : bi * 5 + k + 1],
                                start=(k == 0),
                                stop=(k == 3),
                            )
                    # r = y + t_slack - b  (cast to weight dtype fused)
                    r_mm = tpool.tile([128, hs], wdt, tag=f"rbf{h}")
                    nc.vector.tensor_add(r_mm[:], psum_y[:], tsb[:])

                    # up-projection: psum_u[:, bi*5+j] = As_inv chunk j @ r
                    psum_u = ppool.tile([128, 5 * hs], F32, tag=f"pu{h}")
                    for bi in range(hs):
                        bg = h * hs + bi
                        for j in range(5):
                            nc.tensor.matmul(
                                psum_u[:, bi * 5 + j : bi * 5 + j + 1],
                                lhsT=Pinv_t[bg][:, j * 128 : (j + 1) * 128],
                                rhs=r_mm[:, bi : bi + 1],
                                start=True,
                                stop=True,
                            )
                    # s_new = w - u   (single op: psum_u columns match w layout)
                    s_new = spool.tile([128, hs * 5], F32, tag=f"state{h}")
                    nc.vector.tensor_sub(s_new[:], w[:], psum_u[:])
                    states[h] = s_new
                    if it + 1 < num_iter:
                        preps[h] = emit_prep(h, s_new, float(alphas[it + 1]))

            final = wpool.tile([128, nb * 5], F32, tag="final")
            for h in range(nh):
                nc.scalar.activation(
                    final[:, h * hs * 5 : (h + 1) * hs * 5], states[h][:], AF.Relu
                )
            nc.sync.dma_start(out=out_d[:], in_=final[:])

    return _patch_serialization(nc)


def _prep_core_inputs(c_input, As, bs, As_inv, nb, np_wdt=np.float32):
    """Host-side marshaling of one core's shard into the device layouts."""
    AsT = np.ascontiguousarray(
        As.reshape(nb, 128, 4, 128).transpose(0, 2, 3, 1)
    ).astype(np_wdt)
    Pinv = np.ascontiguousarray(
        As_inv.reshape(nb, 5, 128, 128).transpose(0, 1, 3, 2)
    ).astype(np_wdt)
    ccol = np.ascontiguousarray(
        c_input.reshape(nb, 5, 128).transpose(2, 0, 1).reshape(128, nb * 5),
        dtype=np.float32,
    )
    bcol = np.ascontiguousarray(bs.T, dtype=np.float32)
    return {"AsT": AsT, "Pinv": Pinv, "ccol": ccol, "bcol": bcol}


WEIGHT_DTYPE = "bf16"  # "f32" or "bf16"


def kernel(c_input, As, bs, As_inv, _trace=False, _nc_cache={}):
    import ml_dtypes

    c_input = np.asarray(c_input, dtype=np.float32)
    As = np.asarray(As, dtype=np.float32)
    bs = np.asarray(bs, dtype=np.float32)
    As_inv = np.asarray(As_inv, dtype=np.float32)

    wdt = mybir.dt.bfloat16 if WEIGHT_DTYPE == "bf16" else F32
    np_wdt = ml_dtypes.bfloat16 if WEIGHT_DTYPE == "bf16" else np.float32
    if "nc" not in _nc_cache:
        _nc_cache["nc"] = build_program(wdt=wdt)
    nc = _nc_cache["nc"]

    in_maps = []
    for core in range(NCORES):
        sl = slice(core * NB, (core + 1) * NB)
        in_maps.append(
            _prep_core_inputs(
                c_input[sl], As[sl], bs[sl], As_inv[sl], NB, np_wdt=np_wdt
            )
        )

    res = run_bass_kernel_spmd(nc, in_maps, core_ids=list(range(NCORES)), trace=_trace)

    out = np.empty((B, D), dtype=np.float32)
    for core in range(NCORES):
        oc = res.results[core]["out"]  # [128, NB*5]
        out[core * NB : (core + 1) * NB] = (
            oc.reshape(128, NB, 5).transpose(1, 2, 0).reshape(NB, D)
        )
    if _trace:
        kernel.last_exec_time_ns = res.exec_time_ns
    return out
